# revision 18
# baseline (speedup 1.0000x reference)
"""Trainium2 Bass kernel for a dense transformer block (nn_Block_88338887344891).

Distribution over 8 NeuronCores (single SPMD NEFF, ONE collective):
  - LayerNorm1 is folded into the QKV projection (host-side weight folding:
    W~ = diag(ln_w) @ W, an extra mean-row in the matmul with weights
    -ln_w @ W, and a post bias c~ = ln_b @ W + b).  Each core streams the
    FULL x^T (bf16) from HBM in 512-token chunks, computes per-token
    mean/rsqrt stats itself (replicated, cheap), and produces Q/K/V for its
    2 heads over all 4096 tokens.  This removes the 16.8MB AllGather
    (~265us in the collective cost model) entirely.
  - causal attention per (batch, head) with both heads packed on partitions.
  - attention output AllToAll in bf16 (head-shard -> token-shard), overlapped
    with the FF weight prefetch DMA.
  - output projection + residual + LN2 (folded into FF1 the same way) + MLP
    token-sharded (512 tokens/core), weights streamed in bf16.

Matmuls run in fp8(e4m3) DoubleRow (0.5 cyc/row) where the hardware
Ldweights restrictions allow it (QKV, LN stats, Wo, FF1, FF2 - weights
pre-scaled x32/x64 into the e4m3 normal range, inverse scales folded into
downstream scalar ops), bf16 elsewhere (scores, transposes).
gelu(tanh-approx) is computed as x*sigmoid(1.702x); end-to-end rel err
is ~1.2e-2 against the reference (tolerance 2e-2).
"""
import numpy as np
import ml_dtypes
from contextlib import ExitStack

try:  # persistent XLA cache so repeat runs skip the NEFF compile
    import jax
    jax.config.update("jax_compilation_cache_dir", "/tmp/jax_neff_cache")
    jax.config.update("jax_persistent_cache_min_compile_time_secs", 1.0)
except Exception:
    pass

import concourse.bass as bass
import concourse.bacc as bacc
import concourse.tile as tile
import concourse.mybir as mybir
from concourse.masks import make_identity
from concourse import bass_utils

AF = mybir.ActivationFunctionType
ALU = mybir.AluOpType
F32 = mybir.dt.float32
BF16 = mybir.dt.bfloat16
FP8 = mybir.dt.float8e4
BFNP = ml_dtypes.bfloat16
F8NP = ml_dtypes.float8_e4m3
DR = mybir.MatmulPerfMode.DoubleRow

NC_N = 8          # cores
B, T, D, H = 2, 2048, 1024, 16
BT = B * T        # 4096 tokens total
HD = D // H       # 64
DFF = 4 * D       # 4096
EPS = 1e-5
TPC = BT // NC_N  # 512 tokens per core (output shard)
HPC = H // NC_N   # 2 heads per core
PO = D // 128     # 8 D-tiles
M1 = DFF // 128   # 32 ff1 out tiles
NT = BT // TPC    # 8 token tiles over the full sequence
SIG_A = 1.702     # gelu sigmoid-approx constant
RG = [list(range(NC_N))]

_CACHE = {}


def _build():
    nc = bacc.Bacc("TRN2", target_bir_lowering=False, debug=False,
                   num_devices=NC_N)

    # ---- per-core external inputs (host lays out dram == sbuf layout) ----
    xt_in = nc.dram_tensor("xt", [D, BT], FP8, kind="ExternalInput")
    xo_in = nc.dram_tensor("xo", [128, PO, TPC], BF16, kind="ExternalInput")
    wqk8_in = nc.dram_tensor("wqk8", [128, PO, 256], FP8, kind="ExternalInput")
    wv8_in = nc.dram_tensor("wv8", [128, PO, 128], FP8, kind="ExternalInput")
    uqkv_in = nc.dram_tensor("uqkv", [1, 384], BF16, kind="ExternalInput")
    cqkv_in = nc.dram_tensor("cqkv", [128, 3], F32, kind="ExternalInput")
    wo_in = nc.dram_tensor("wo", [128, PO, PO, 128], FP8, kind="ExternalInput")
    bo_in = nc.dram_tensor("bo", [128, PO], F32, kind="ExternalInput")
    wf1_in = nc.dram_tensor("wf1", [M1, 128, PO, 128], FP8, kind="ExternalInput")
    cf1_in = nc.dram_tensor("cf1", [128, M1], F32, kind="ExternalInput")
    cf1s_in = nc.dram_tensor("cf1s", [128, M1], F32, kind="ExternalInput")
    wf2_in = nc.dram_tensor("wf2", [PO, 128, M1, 128], FP8, kind="ExternalInput")
    cf2_in = nc.dram_tensor("cf2", [128, PO], F32, kind="ExternalInput")
    mask_in = nc.dram_tensor("trimask", [128, 4, TPC], FP8, kind="ExternalInput")
    out_t = nc.dram_tensor("outt", [D, TPC], F32, kind="ExternalOutput")

    with tile.TileContext(nc) as tc, ExitStack() as ctx:
        perm = ctx.enter_context(tc.tile_pool(name="perm", bufs=1))
        rows = ctx.enter_context(tc.tile_pool(name="rows", bufs=3))
        psA = ctx.enter_context(tc.tile_pool(name="psA", bufs=2, space="PSUM"))
        dram = ctx.enter_context(tc.tile_pool(name="dram", bufs=1, space="DRAM"))

        # ---- constants ----
        ones_col_b = perm.tile([128, 1], BF16)
        nc.vector.memset(ones_col_b[:], 1.0)
        invd_col_b = perm.tile([128, 1], BF16)
        nc.vector.memset(invd_col_b[:], 1.0 / D)
        ones8_col = perm.tile([128, 1], FP8)
        nc.vector.memset(ones8_col[:], 1.0)
        ones8_pair = perm.tile([128, 2, 16], FP8)
        nc.vector.memset(ones8_pair[:], 1.0)
        ident_f = perm.tile([128, 128], F32)
        make_identity(nc, ident_f[:])
        ident_b = perm.tile([128, 128], BF16)
        nc.vector.tensor_copy(ident_b[:], ident_f[:])

        def load_const(t_in, shape, tag, dt=F32):
            t = perm.tile(shape, dt, tag=tag)
            nc.sync.dma_start(t[:], t_in.ap())
            return t

        # first x chunk load goes out before the small consts so the PE
        # pipeline fills as early as possible
        xview0 = xt_in.ap().rearrange("(po p) (nt t) -> p po nt t",
                                      p=128, nt=NT)
        Xc0 = perm.tile([128, PO, TPC], FP8, tag="xc0")
        nc.sync.dma_start(Xc0[:], xview0[:, :, 0, :])

        cqkv = load_const(cqkv_in, [128, 3], "c_cqkv")
        uqkv = load_const(uqkv_in, [1, 384], "c_uqkv", BF16)
        bo = load_const(bo_in, [128, PO], "c_bo")
        cf1 = load_const(cf1_in, [128, M1], "c_cf1")
        cf1s = load_const(cf1s_in, [128, M1], "c_cf1s")
        cf2 = load_const(cf2_in, [128, PO], "c_cf2")

        def row_stats(ps_st, ps_sq, inv_scale=1.0, mean_scale=1.0):
            """ps_st/ps_sq [1,TPC] psum: sum and sum-of-squares over D.
            Returns (mu_b bf16 [1,TPC], inv_bc bf16 [128,TPC])."""
            mu_b = rows.tile([1, TPC], BF16, tag="mub")
            nc.scalar.activation(mu_b[:], ps_st[0:1, :], AF.Copy,
                                 scale=mean_scale)
            sqmu = rows.tile([1, TPC], F32, tag="sqm")
            nc.scalar.activation(sqmu[:], mu_b[:], AF.Square)
            var = rows.tile([1, TPC], F32, tag="var")
            nc.vector.scalar_tensor_tensor(
                out=var[:], in0=ps_sq[0:1, :], scalar=mean_scale,
                in1=sqmu[:], op0=ALU.mult, op1=ALU.subtract)
            rec = rows.tile([1, TPC], F32, tag="rec")
            nc.vector.reciprocal(rec[:], var[:])
            inv_b = rows.tile([1, TPC], BF16, tag="inv")
            nc.scalar.activation(inv_b[:], rec[:], AF.Sqrt, scale=inv_scale)
            inv_bc = rows.tile([128, TPC], BF16, tag="invbc")
            nc.gpsimd.partition_broadcast(inv_bc[:], inv_b[:])
            return mu_b, inv_bc

        mlp = ctx.enter_context(tc.tile_pool(name="mlp", bufs=1))

        with tc.tile_pool(name="attnw", bufs=1) as attnw:
            # ============== Phase 1: streamed LN1-folded QKV ================
            QT = attnw.tile([128, NT, TPC], BF16)
            KT = attnw.tile([128, NT, TPC], BF16)
            Vt = attnw.tile([128, 2 * NT * HPC, HPC, 80], FP8)
            nc.vector.tensor_copy(Vt[:, :, :, 64:65],
                                  ones8_col[:].to_broadcast([128, 32, HPC, 1]))
            trimask = attnw.tile([128, 4, TPC], FP8)
            nc.sync.dma_start(trimask[:], mask_in.ap())
            wqk8_sb = attnw.tile([128, PO, 256], FP8)
            nc.sync.dma_start(wqk8_sb[:], wqk8_in.ap())
            wv8_sb = attnw.tile([128, PO, 128], FP8)
            nc.sync.dma_start(wv8_sb[:], wv8_in.ap())

            xview = xt_in.ap().rearrange("(po p) (nt t) -> p po nt t",
                                         p=128, nt=NT)
            a2ai = dram.tile([NC_N, 128, TPC], FP8)
            a2ao = dram.tile([NC_N, 128, TPC], FP8)
            with tc.tile_pool(name="xp", bufs=3) as xp, \
                 tc.tile_pool(name="sqp", bufs=3) as sqp, \
                 tc.tile_pool(name="vtp", bufs=3) as vtp, \
                 tc.tile_pool(name="ptp", bufs=2) as ptp, \
                 tc.tile_pool(name="avp", bufs=3) as avp, \
                 tc.tile_pool(name="aps", bufs=1, space="PSUM") as aps:

                def do_chunk(tt):
                    if tt == 0:
                        Xc = Xc0
                    else:
                        Xc = xp.tile([128, PO, TPC], FP8, tag="xc")
                        nc.sync.dma_start(Xc[:], xview[:, :, tt, :])
                    sq = sqp.tile([128, PO, TPC], FP8, tag="sq")
                    for po in range(PO):  # split squares across ACT/DVE/Pool
                        if po % 8 < 2:
                            nc.scalar.activation(sq[:, po, :], Xc[:, po, :],
                                                 AF.Square)
                        elif po % 8 < 4:
                            nc.vector.tensor_mul(sq[:, po, :], Xc[:, po, :],
                                                 Xc[:, po, :])
                        else:
                            nc.gpsimd.tensor_mul(sq[:, po, :], Xc[:, po, :],
                                                 Xc[:, po, :])
                    ps_st = aps.tile([1, TPC], F32, tag="st")
                    ps_sq = aps.tile([1, TPC], F32, tag="stq")
                    for q in range(PO // 2):
                        nc.tensor.matmul(ps_st[0:1, :],
                                         ones8_pair[:, :, 0:1],
                                         Xc[:, 2 * q:2 * q + 2, :],
                                         start=(q == 0),
                                         stop=(q == PO // 2 - 1), perf_mode=DR)
                    for q in range(PO // 2):
                        nc.tensor.matmul(ps_sq[0:1, :],
                                         ones8_pair[:, :, 0:1],
                                         sq[:, 2 * q:2 * q + 2, :],
                                         start=(q == 0),
                                         stop=(q == PO // 2 - 1), perf_mode=DR)
                    mu_b, inv_bc = row_stats(ps_st, ps_sq,
                                             inv_scale=1.0 / (32 * 32),
                                             mean_scale=1.0 / D)

                    for blk in range(3):
                        ps = psA.tile([128, TPC], F32, tag="ps")
                        if blk < 2:
                            for q in range(PO // 2):
                                nc.tensor.matmul(
                                    ps[:],
                                    wqk8_sb[:, 2 * q:2 * q + 2,
                                            128 * blk:128 * blk + 128],
                                    Xc[:, 2 * q:2 * q + 2, :],
                                    start=(q == 0), stop=False, perf_mode=DR)
                        else:
                            for q in range(PO // 2):
                                nc.tensor.matmul(
                                    ps[:], wv8_sb[:, 2 * q:2 * q + 2, :],
                                    Xc[:, 2 * q:2 * q + 2, :],
                                    start=(q == 0), stop=False, perf_mode=DR)
                        nc.tensor.matmul(
                            ps[:], uqkv[0:1, 128 * blk:128 * blk + 128],
                            mu_b[:], start=False, stop=True)
                        if blk < 2:
                            DST = (QT, KT)[blk]
                            nc.vector.tensor_mul(DST[:, tt, :], ps[:],
                                                 inv_bc[:])
                            nc.vector.tensor_scalar_add(
                                DST[:, tt, :], DST[:, tt, :],
                                cqkv[:, blk:blk + 1])
                        else:
                            vt_t = vtp.tile([128, TPC], BF16, tag="vtt")
                            nc.vector.tensor_mul(vt_t[:], ps[:], inv_bc[:])
                            nc.vector.tensor_scalar_add(vt_t[:], vt_t[:],
                                                        cqkv[:, 2:3])
                            pstt = psA.tile([128, TPC], BF16, tag="ps")
                            for q4 in range(4):
                                nc.tensor.transpose(
                                    pstt[:, 128 * q4:128 * q4 + 128],
                                    vt_t[:, 128 * q4:128 * q4 + 128],
                                    ident_b[:])
                            for q4 in range(4):
                                g = 4 * tt + q4
                                pv = pstt[:, 128 * q4:128 * q4 + 128].rearrange(
                                    "p (h d) -> p h d", h=HPC)
                                if q4 % 2 == 0:
                                    nc.vector.tensor_copy(Vt[:, g, :, 0:64], pv)
                                else:
                                    nc.scalar.activation(Vt[:, g, :, 0:64], pv,
                                                         AF.Copy)

                # ===== Phase 2: causal attention per (head, batch) ======
                def do_block(b, j):
                        n_kt = 4 * j + 4
                        PT = ptp.tile([128, 16, 2 * TPC], FP8, tag="pt")
                        for i in range(n_kt):
                            pss = aps.tile([128, 2 * TPC], F32, tag="ps2", bufs=2)
                            cb = 4 * b + i // 4
                            off = (i % 4) * 128
                            # diagonal tiles: queries below 128*d are fully
                            # masked, so skip them in the matmul/exp/mask and
                            # just zero that strip of PT
                            d = i - 4 * j
                            q0 = 128 * d if d > 0 else 0
                            PTv = PT[:, i, :].rearrange("p (h q) -> p h q",
                                                        h=HPC)
                            pssv = pss[:].rearrange("p (h q) -> p h q", h=HPC)
                            for h in range(HPC):
                                nc.tensor.matmul(
                                    pss[:, h * TPC + q0:(h + 1) * TPC],
                                    KT[64 * h:64 * h + 64, cb, off:off + 128],
                                    QT[64 * h:64 * h + 64, 4 * b + j, q0:],
                                    start=True, stop=True)
                            if q0:
                                nc.gpsimd.memset(PTv[:, :, 0:q0], 0.0)
                            nc.scalar.activation(PTv[:, :, q0:],
                                                 pssv[:, :, q0:],
                                                 AF.Exp, scale=0.125)
                            if i >= 4 * j:
                                eng = nc.vector if i % 2 == 0 else nc.gpsimd
                                eng.tensor_mul(
                                    PTv[:, :, q0:], PTv[:, :, q0:],
                                    trimask[:, d:d + 1, q0:].to_broadcast(
                                        [128, HPC, TPC - q0]))
                        for h in range(HPC):
                            ps_av = psA.tile([65, TPC], F32, tag="ps")
                            PTr = PT[:].rearrange("p i (h q) -> p i h q",
                                                  h=HPC)
                            for q in range(n_kt // 2):
                                nc.tensor.matmul(
                                    ps_av[:],
                                    Vt[:, 16 * b + 2 * q:16 * b + 2 * q + 2,
                                       h, 0:65],
                                    PTr[:, 2 * q:2 * q + 2, h, :],
                                    start=(q == 0), stop=(q == n_kt // 2 - 1),
                                    perf_mode=DR)
                            rec = avp.tile([1, TPC], F32, tag="avrec")
                            nc.vector.reciprocal(rec[:], ps_av[64:65, :])
                            recb = avp.tile([1, TPC], BF16, tag="avrecb")
                            nc.vector.tensor_copy(recb[:], rec[:])
                            rb = avp.tile([64, TPC], BF16, tag="avrb")
                            nc.gpsimd.partition_broadcast(rb[:], recb[:])
                            avn = avp.tile([64, TPC], FP8, tag="avn")
                            nc.vector.tensor_mul(avn[:], ps_av[0:64, :], rb[:])
                            nc.sync.dma_start(
                                a2ai[4 * b + j, 64 * h:64 * h + 64, :], avn[:])

                # attention block (b,j) needs only chunks <= 4b+j, so
                # interleave 1:1 with the QKV stream - exps start right
                # after chunk 0 and fill the ACT engine end to end
                for t in range(NT):
                    do_chunk(t)
                    if t == NT - 1:
                        xown = attnw.tile([128, PO, TPC], BF16)
                        nc.sync.dma_start(xown[:], xo_in.ap())
                        wo_sb = attnw.tile([128, PO, PO, 128], FP8)
                        nc.sync.dma_start(wo_sb[:], wo_in.ap())
                    do_block(t // 4, t % 4)

            # ==== Phase 3: AllToAll (overlapped with FF weight prefetch) ====
            nc.gpsimd.collective_compute(
                "AllToAll", ALU.bypass, replica_groups=RG,
                ins=[a2ai[:].opt()], outs=[a2ao[:].opt()])
            AVt = attnw.tile([128, NC_N, TPC], FP8)
            nc.sync.dma_start(AVt[:], a2ao[:].rearrange("s p t -> p s t"))

            # ========== Phase 4: output projection + residual ===============
            X2 = mlp.tile([128, PO, TPC], BF16)
            for m in range(PO):
                ps_o = psA.tile([128, TPC], F32, tag="ps")
                for q in range(PO // 2):
                    nc.tensor.matmul(ps_o[:], wo_sb[:, m, 2 * q:2 * q + 2, :],
                                     AVt[:, 2 * q:2 * q + 2, :],
                                     start=(q == 0), stop=(q == PO // 2 - 1),
                                     perf_mode=DR)
                to = rows.tile([128, TPC], BF16, tag="to")
                nc.scalar.activation(to[:], ps_o[:], AF.Identity,
                                     scale=1.0 / 32, bias=bo[:, m:m + 1])
                nc.vector.tensor_add(X2[:, m, :], to[:], xown[:, m, :])

        # ================= Phase 5: LN2-folded MLP ==========================
        A = mlp.tile([128, M1, TPC], FP8)
        with tc.tile_pool(name="w1p", bufs=4) as w1p, \
             tc.tile_pool(name="w2p", bufs=3) as w2p, \
             tc.tile_pool(name="sq2p", bufs=1) as sq2p, \
             tc.tile_pool(name="pst2", bufs=1, space="PSUM") as pstat2, \
             tc.tile_pool(name="psM", bufs=2, space="PSUM") as psM, \
             tc.tile_pool(name="gp", bufs=3) as gp, \
             tc.tile_pool(name="outp", bufs=3) as outp:
            sq2 = sq2p.tile([128, PO, TPC], BF16)
            for po in range(PO):
                if po % 2 == 0:
                    nc.gpsimd.tensor_mul(sq2[:, po, :], X2[:, po, :],
                                         X2[:, po, :])
                else:
                    nc.scalar.activation(sq2[:, po, :], X2[:, po, :],
                                         AF.Square)
            ps_st2 = pstat2.tile([1, TPC], F32, tag="st")
            ps_sq2 = pstat2.tile([1, TPC], F32, tag="stq")
            for po in range(PO):
                nc.tensor.matmul(ps_st2[0:1, :], invd_col_b[:], X2[:, po, :],
                                 start=(po == 0), stop=(po == PO - 1))
            for po in range(PO):
                nc.tensor.matmul(ps_sq2[0:1, :], invd_col_b[:], sq2[:, po, :],
                                 start=(po == 0), stop=(po == PO - 1))
            mu2_b, inv2_bc = row_stats(ps_st2, ps_sq2,
                                       inv_scale=1.0 / (32 * 32))
            mu2_bc = rows.tile([128, TPC], BF16, tag="mubc")
            nc.gpsimd.partition_broadcast(mu2_bc[:], mu2_b[:])
            # X2c = (X2 - mu2) * inv2 / 32  (the per-token LN2 scale rides the
            # matmul columns, so FF1 psum needs no per-token fixup at all)
            X2c = mlp.tile([128, PO, TPC], FP8)
            ct = gp.tile([128, PO, TPC], BF16, tag="ct")
            for po in range(PO):
                eng = nc.vector if po % 2 == 0 else nc.gpsimd
                eng.tensor_sub(ct[:, po, :], X2[:, po, :], mu2_bc[:])
                eng.tensor_mul(X2c[:, po, :], ct[:, po, :], inv2_bc[:])

            for m in range(M1):
                w1m = w1p.tile([128, PO, 128], FP8, tag="w1")
                nc.sync.dma_start(w1m[:], wf1_in.ap()[m])
                if m % 2 == 0:
                    ps1 = psA.tile([128, TPC], F32, tag="ps")
                else:
                    ps1 = psM.tile([128, TPC], F32, tag="psm")
                for q in range(PO // 2):
                    nc.tensor.matmul(ps1[:], w1m[:, 2 * q:2 * q + 2, :],
                                     X2c[:, 2 * q:2 * q + 2, :],
                                     start=(q == 0), stop=(q == PO // 2 - 1),
                                     perf_mode=DR)
                sig = gp.tile([128, TPC], BF16, tag="sig")
                nc.scalar.activation(sig[:], ps1[:], AF.Sigmoid,
                                     scale=SIG_A, bias=cf1s[:, m:m + 1])
                nc.vector.scalar_tensor_tensor(
                    out=A[:, m, :], in0=ps1[:], scalar=cf1[:, m:m + 1],
                    in1=sig[:], op0=ALU.add, op1=ALU.mult)

            out_view = out_t.ap().rearrange("(po p) t -> p po t", p=128)
            for m in range(PO):
                w2m = w2p.tile([128, M1, 128], FP8, tag="w2")
                nc.sync.dma_start(w2m[:], wf2_in.ap()[m])
                if m % 2 == 0:
                    ps_2 = psA.tile([128, TPC], F32, tag="ps")
                else:
                    ps_2 = psM.tile([128, TPC], F32, tag="psm")
                for q in range(M1 // 2):
                    nc.tensor.matmul(ps_2[:], w2m[:, 2 * q:2 * q + 2, :],
                                     A[:, 2 * q:2 * q + 2, :],
                                     start=(q == 0), stop=(q == M1 // 2 - 1),
                                     perf_mode=DR)
                t2 = gp.tile([128, TPC], F32, tag="t2")
                nc.scalar.activation(t2[:], ps_2[:], AF.Identity,
                                     scale=1.0 / 64, bias=cf2[:, m:m + 1])
                om = outp.tile([128, TPC], F32, tag="om")
                eng = nc.vector if m % 2 == 0 else nc.gpsimd
                eng.tensor_add(om[:], t2[:], X2[:, m, :])
                nc.sync.dma_start(out_view[:, m, :], om[:])

    nc.compile()
    return nc


def _get_nc():
    if "nc" not in _CACHE:
        _CACHE["nc"] = _build()
    return _CACHE["nc"]


def _make_in_maps(inputs):
    f32 = np.float32
    x = np.asarray(inputs["x"], f32).reshape(BT, D)
    ln1w = np.asarray(inputs["ln1_w"], f32)
    ln1b = np.asarray(inputs["ln1_b"], f32)
    ln2w = np.asarray(inputs["ln2_w"], f32)
    ln2b = np.asarray(inputs["ln2_b"], f32)
    W_qkv = np.asarray(inputs["W_qkv"], f32)
    b_qkv = np.asarray(inputs["b_qkv"], f32)
    W_o = np.asarray(inputs["W_o"], f32)
    b_o = np.asarray(inputs["b_o"], f32)
    W_ff1 = np.asarray(inputs["W_ff1"], f32)
    b_ff1 = np.asarray(inputs["b_ff1"], f32)
    W_ff2 = np.asarray(inputs["W_ff2"], f32)
    b_ff2 = np.asarray(inputs["b_ff2"], f32)

    # LN1 folded into QKV
    Wq_t = W_qkv * ln1w[:, None]            # [D, 3D]
    u_q = ln1w @ W_qkv                      # [3D]
    c_q = ln1b @ W_qkv + b_qkv              # [3D]
    # LN2 folded into FF1
    Wf1_t = W_ff1 * ln2w[:, None]           # [D, DFF]
    u_f = ln2w @ W_ff1                      # [DFF]
    c_f = ln2b @ W_ff1 + b_ff1              # [DFF]

    xt_full = np.ascontiguousarray(x.T).astype(BFNP)      # [D, BT]
    xt_full8 = xt_full.astype(F8NP)

    def pcol(v):  # [K*128] -> [128, K]
        return np.ascontiguousarray(v.reshape(-1, 128).T.astype(f32))

    # causal masks for the 4 diagonal sub-tiles: keep where tau >= 128*d + p
    p = np.arange(128)[:, None]
    tau = np.arange(TPC)[None, :]
    trimask = np.stack([(tau >= 128 * d + p) for d in range(4)],
                       axis=1).astype(F8NP)               # [128, 4, TPC]

    common = {
        "xt": xt_full8,
        "wo": np.ascontiguousarray(
            32.0 * W_o.reshape(PO, 128, PO, 128).transpose(1, 2, 0, 3)
        ).astype(F8NP),
        "bo": pcol(b_o),
        "wf1": np.ascontiguousarray(
            32.0 * Wf1_t.reshape(PO, 128, M1, 128).transpose(2, 1, 0, 3)
        ).astype(F8NP),
        "cf1": pcol(c_f),
        "cf1s": pcol(SIG_A * c_f),
        "wf2": np.ascontiguousarray(
            64.0 * W_ff2.reshape(M1, 128, PO, 128).transpose(2, 1, 0, 3)
        ).astype(F8NP),
        "cf2": pcol(b_ff2),
        "trimask": trimask,
    }
    in_maps = []
    for r in range(NC_N):
        hc = 128 * r          # first column of this core's Q/K/V head block
        m = dict(common)
        wqk = np.concatenate([Wq_t[:, hc:hc + 128],
                              Wq_t[:, D + hc:D + hc + 128]], axis=1)
        m["wqk8"] = np.ascontiguousarray(
            32.0 * wqk.reshape(PO, 128, 256).transpose(1, 0, 2)).astype(F8NP)
        m["wv8"] = np.ascontiguousarray(
            32.0 * Wq_t[:, 2 * D + hc:2 * D + hc + 128]
            .reshape(PO, 128, 128).transpose(1, 0, 2)).astype(F8NP)
        m["uqkv"] = np.ascontiguousarray(-32.0 * np.concatenate(
            [u_q[hc:hc + 128], u_q[D + hc:D + hc + 128],
             u_q[2 * D + hc:2 * D + hc + 128]])[None, :]).astype(BFNP)
        m["cqkv"] = np.ascontiguousarray(np.stack(
            [c_q[hc:hc + 128], c_q[D + hc:D + hc + 128],
             c_q[2 * D + hc:2 * D + hc + 128]], axis=1)).astype(f32)
        m["xo"] = np.ascontiguousarray(
            xt_full[:, TPC * r:TPC * (r + 1)].reshape(PO, 128, TPC)
            .transpose(1, 0, 2))
        in_maps.append(m)
    return in_maps


def _run(inputs, trace=False, trace_cores=None):
    nc = _get_nc()
    in_maps = _make_in_maps(inputs)
    res = bass_utils.run_bass_kernel_spmd(
        nc, in_maps, core_ids=list(range(NC_N)), trace=trace,
        trace_cores=trace_cores)
    outs = [res.results[r]["outt"] for r in range(NC_N)]
    full = np.concatenate([np.asarray(o, np.float32).T for o in outs], axis=0)
    return full.reshape(B, T, D).astype(np.float32), res


def kernel(**inputs):
    out, _ = _run(inputs, trace=False)
    return out


# revision 22
# speedup vs baseline: 1.1125x; 1.1125x over previous
"""Trainium2 Bass kernel for a dense transformer block (nn_Block_88338887344891).

Distribution over 8 NeuronCores (single SPMD NEFF, ONE collective):
  - LayerNorm1 is folded into the QKV projection (host-side weight folding:
    W~ = diag(ln_w) @ W, an extra mean-row in the matmul with weights
    -ln_w @ W, and a post bias c~ = ln_b @ W + b).  Each core streams the
    FULL x^T (bf16) from HBM in 512-token chunks, computes per-token
    mean/rsqrt stats itself (replicated, cheap), and produces Q/K/V for its
    2 heads over all 4096 tokens.  This removes the 16.8MB AllGather
    (~265us in the collective cost model) entirely.
  - causal attention per (batch, head) with both heads packed on partitions.
  - attention output AllToAll in bf16 (head-shard -> token-shard), overlapped
    with the FF weight prefetch DMA.
  - output projection + residual + LN2 (folded into FF1 the same way) + MLP
    token-sharded (512 tokens/core), weights streamed in bf16.

Matmuls run in fp8(e4m3) DoubleRow (0.5 cyc/row) where the hardware
Ldweights restrictions allow it (QKV, LN stats, Wo, FF1, FF2 - weights
pre-scaled x32/x64 into the e4m3 normal range, inverse scales folded into
downstream scalar ops), bf16 elsewhere (scores, transposes).
gelu(tanh-approx) is computed as x*sigmoid(1.702x); end-to-end rel err
is ~1.2e-2 against the reference (tolerance 2e-2).
"""
import numpy as np
import ml_dtypes
from contextlib import ExitStack

try:  # persistent XLA cache so repeat runs skip the NEFF compile
    import jax
    jax.config.update("jax_compilation_cache_dir", "/tmp/jax_neff_cache")
    jax.config.update("jax_persistent_cache_min_compile_time_secs", 1.0)
except Exception:
    pass

import concourse.bass as bass
import concourse.bacc as bacc
import concourse.tile as tile
import concourse.mybir as mybir
from concourse.masks import make_identity
from concourse import bass_utils

AF = mybir.ActivationFunctionType
ALU = mybir.AluOpType
F32 = mybir.dt.float32
BF16 = mybir.dt.bfloat16
FP8 = mybir.dt.float8e4
BFNP = ml_dtypes.bfloat16
F8NP = ml_dtypes.float8_e4m3
DR = mybir.MatmulPerfMode.DoubleRow

NC_N = 8          # cores
B, T, D, H = 2, 2048, 1024, 16
BT = B * T        # 4096 tokens total
HD = D // H       # 64
DFF = 4 * D       # 4096
EPS = 1e-5
TPC = BT // NC_N  # 512 tokens per core (output shard)
HPC = H // NC_N   # 2 heads per core
PO = D // 128     # 8 D-tiles
M1 = DFF // 128   # 32 ff1 out tiles
NT = BT // TPC    # 8 token tiles over the full sequence
SIG_A = 1.702     # gelu sigmoid-approx constant
RG = [list(range(NC_N))]

_CACHE = {}


def _build():
    nc = bacc.Bacc("TRN2", target_bir_lowering=False, debug=False,
                   num_devices=NC_N)

    # ---- per-core external inputs (host lays out dram == sbuf layout) ----
    xt_in = nc.dram_tensor("xt", [D, BT], FP8, kind="ExternalInput")
    xo_in = nc.dram_tensor("xo", [128, PO, TPC], BF16, kind="ExternalInput")
    wqk8_in = nc.dram_tensor("wqk8", [128, PO, 256], FP8, kind="ExternalInput")
    wv8_in = nc.dram_tensor("wv8", [128, PO, 128], FP8, kind="ExternalInput")
    uqkv_in = nc.dram_tensor("uqkv", [1, 384], BF16, kind="ExternalInput")
    cqkv_in = nc.dram_tensor("cqkv", [128, 3], F32, kind="ExternalInput")
    wo_in = nc.dram_tensor("wo", [128, PO, PO, 128], FP8, kind="ExternalInput")
    bo_in = nc.dram_tensor("bo", [128, PO], F32, kind="ExternalInput")
    wf1_in = nc.dram_tensor("wf1", [M1, 128, PO, 128], FP8, kind="ExternalInput")
    cf1_in = nc.dram_tensor("cf1", [128, M1], F32, kind="ExternalInput")
    cf1s_in = nc.dram_tensor("cf1s", [128, M1], F32, kind="ExternalInput")
    wf2_in = nc.dram_tensor("wf2", [PO, 128, M1, 128], FP8, kind="ExternalInput")
    cf2_in = nc.dram_tensor("cf2", [128, PO], F32, kind="ExternalInput")
    mask_in = nc.dram_tensor("trimask", [128, 4, TPC], FP8, kind="ExternalInput")
    out_t = nc.dram_tensor("outt", [D, TPC], F32, kind="ExternalOutput")

    with tile.TileContext(nc) as tc, ExitStack() as ctx:
        perm = ctx.enter_context(tc.tile_pool(name="perm", bufs=1))
        rows = ctx.enter_context(tc.tile_pool(name="rows", bufs=3))
        psA = ctx.enter_context(tc.tile_pool(name="psA", bufs=2, space="PSUM"))
        dram = ctx.enter_context(tc.tile_pool(name="dram", bufs=1, space="DRAM"))

        # ---- constants ----
        ones_col_b = perm.tile([128, 1], BF16)
        nc.vector.memset(ones_col_b[:], 1.0)
        invd_col_b = perm.tile([128, 1], BF16)
        nc.vector.memset(invd_col_b[:], 1.0 / D)
        ones8_col = perm.tile([128, 1], FP8)
        nc.vector.memset(ones8_col[:], 1.0)
        ones8_pair = perm.tile([128, 2, 16], FP8)
        nc.vector.memset(ones8_pair[:], 1.0)
        ident_f = perm.tile([128, 128], F32)
        make_identity(nc, ident_f[:])
        ident_b = perm.tile([128, 128], BF16)
        nc.vector.tensor_copy(ident_b[:], ident_f[:])

        def load_const(t_in, shape, tag, dt=F32):
            t = perm.tile(shape, dt, tag=tag)
            nc.sync.dma_start(t[:], t_in.ap())
            return t

        # first x chunk load goes out before the small consts so the PE
        # pipeline fills as early as possible
        xview0 = xt_in.ap().rearrange("(po p) (nt t) -> p po nt t",
                                      p=128, nt=NT)
        Xc0 = perm.tile([128, PO, TPC], FP8, tag="xc0")
        nc.sync.dma_start(Xc0[:], xview0[:, :, 0, :])

        cqkv = load_const(cqkv_in, [128, 3], "c_cqkv")
        uqkv = load_const(uqkv_in, [1, 384], "c_uqkv", BF16)
        bo = load_const(bo_in, [128, PO], "c_bo")
        cf1 = load_const(cf1_in, [128, M1], "c_cf1")
        cf1s = load_const(cf1s_in, [128, M1], "c_cf1s")
        cf2 = load_const(cf2_in, [128, PO], "c_cf2")

        def row_stats(ps_st, ps_sq, inv_scale=1.0, mean_scale=1.0):
            """ps_st/ps_sq [1,TPC] psum: sum and sum-of-squares over D.
            Returns (mu_b bf16 [1,TPC], inv_bc bf16 [128,TPC])."""
            mu_b = rows.tile([1, TPC], BF16, tag="mub")
            nc.scalar.activation(mu_b[:], ps_st[0:1, :], AF.Copy,
                                 scale=mean_scale)
            sqmu = rows.tile([1, TPC], F32, tag="sqm")
            nc.scalar.activation(sqmu[:], mu_b[:], AF.Square)
            var = rows.tile([1, TPC], F32, tag="var")
            nc.vector.scalar_tensor_tensor(
                out=var[:], in0=ps_sq[0:1, :], scalar=mean_scale,
                in1=sqmu[:], op0=ALU.mult, op1=ALU.subtract)
            rec = rows.tile([1, TPC], F32, tag="rec")
            nc.vector.reciprocal(rec[:], var[:])
            inv_b = rows.tile([1, TPC], BF16, tag="inv")
            nc.scalar.activation(inv_b[:], rec[:], AF.Sqrt, scale=inv_scale)
            inv_bc = rows.tile([128, TPC], BF16, tag="invbc")
            nc.gpsimd.partition_broadcast(inv_bc[:], inv_b[:])
            return mu_b, inv_bc

        mlp = ctx.enter_context(tc.tile_pool(name="mlp", bufs=1))

        with tc.tile_pool(name="attnw", bufs=1) as attnw:
            # ============== Phase 1: streamed LN1-folded QKV ================
            QT = attnw.tile([128, NT, TPC], BF16)
            KT = attnw.tile([128, NT, TPC], BF16)
            Vt = attnw.tile([128, 2 * NT * HPC, HPC, 80], FP8)
            nc.vector.tensor_copy(Vt[:, :, :, 64:65],
                                  ones8_col[:].to_broadcast([128, 32, HPC, 1]))
            trimask = attnw.tile([128, 4, TPC], FP8)
            nc.sync.dma_start(trimask[:], mask_in.ap())
            wqk8_sb = attnw.tile([128, PO, 256], FP8)
            nc.sync.dma_start(wqk8_sb[:], wqk8_in.ap())
            wv8_sb = attnw.tile([128, PO, 128], FP8)
            nc.sync.dma_start(wv8_sb[:], wv8_in.ap())

            xview = xt_in.ap().rearrange("(po p) (nt t) -> p po nt t",
                                         p=128, nt=NT)
            a2ai = dram.tile([NC_N, 128, TPC], FP8)
            a2ao = dram.tile([NC_N, 128, TPC], FP8)
            with tc.tile_pool(name="xp", bufs=3) as xp, \
                 tc.tile_pool(name="sqp", bufs=3) as sqp, \
                 tc.tile_pool(name="vtp", bufs=3) as vtp, \
                 tc.tile_pool(name="ptp", bufs=2) as ptp, \
                 tc.tile_pool(name="avp", bufs=3) as avp, \
                 tc.tile_pool(name="aps", bufs=1, space="PSUM") as aps:

                def do_chunk(tt):
                    if tt == 0:
                        Xc = Xc0
                    else:
                        Xc = xp.tile([128, PO, TPC], FP8, tag="xc")
                        nc.sync.dma_start(Xc[:], xview[:, :, tt, :])
                    sq = sqp.tile([128, PO, TPC], FP8, tag="sq")
                    for po in range(PO):  # split squares across ACT/DVE/Pool
                        if po % 8 < 2:
                            nc.scalar.activation(sq[:, po, :], Xc[:, po, :],
                                                 AF.Square)
                        elif po % 8 < 4:
                            nc.vector.tensor_mul(sq[:, po, :], Xc[:, po, :],
                                                 Xc[:, po, :])
                        else:
                            nc.gpsimd.tensor_mul(sq[:, po, :], Xc[:, po, :],
                                                 Xc[:, po, :])
                    ps_st = aps.tile([1, TPC], F32, tag="st")
                    ps_sq = aps.tile([1, TPC], F32, tag="stq")
                    for q in range(PO // 2):
                        nc.tensor.matmul(ps_st[0:1, :],
                                         ones8_pair[:, :, 0:1],
                                         Xc[:, 2 * q:2 * q + 2, :],
                                         start=(q == 0),
                                         stop=(q == PO // 2 - 1), perf_mode=DR)
                    for q in range(PO // 2):
                        nc.tensor.matmul(ps_sq[0:1, :],
                                         ones8_pair[:, :, 0:1],
                                         sq[:, 2 * q:2 * q + 2, :],
                                         start=(q == 0),
                                         stop=(q == PO // 2 - 1), perf_mode=DR)
                    mu_b, inv_bc = row_stats(ps_st, ps_sq,
                                             inv_scale=1.0 / (32 * 32),
                                             mean_scale=1.0 / D)

                    for blk in range(3):
                        ps = psA.tile([128, TPC], F32, tag="ps")
                        if blk < 2:
                            for q in range(PO // 2):
                                nc.tensor.matmul(
                                    ps[:],
                                    wqk8_sb[:, 2 * q:2 * q + 2,
                                            128 * blk:128 * blk + 128],
                                    Xc[:, 2 * q:2 * q + 2, :],
                                    start=(q == 0), stop=False, perf_mode=DR)
                        else:
                            for q in range(PO // 2):
                                nc.tensor.matmul(
                                    ps[:], wv8_sb[:, 2 * q:2 * q + 2, :],
                                    Xc[:, 2 * q:2 * q + 2, :],
                                    start=(q == 0), stop=False, perf_mode=DR)
                        nc.tensor.matmul(
                            ps[:], uqkv[0:1, 128 * blk:128 * blk + 128],
                            mu_b[:], start=False, stop=True)
                        if blk < 2:
                            DST = (QT, KT)[blk]
                            nc.vector.tensor_mul(DST[:, tt, :], ps[:],
                                                 inv_bc[:])
                            nc.vector.tensor_scalar_add(
                                DST[:, tt, :], DST[:, tt, :],
                                cqkv[:, blk:blk + 1])
                        else:
                            vt_t = vtp.tile([128, TPC], BF16, tag="vtt")
                            nc.vector.tensor_mul(vt_t[:], ps[:], inv_bc[:])
                            nc.vector.tensor_scalar_add(vt_t[:], vt_t[:],
                                                        cqkv[:, 2:3])
                            pstt = psA.tile([128, TPC], BF16, tag="ps")
                            for q4 in range(4):
                                nc.tensor.transpose(
                                    pstt[:, 128 * q4:128 * q4 + 128],
                                    vt_t[:, 128 * q4:128 * q4 + 128],
                                    ident_b[:])
                            for q4 in range(4):
                                g = 4 * tt + q4
                                pv = pstt[:, 128 * q4:128 * q4 + 128].rearrange(
                                    "p (h d) -> p h d", h=HPC)
                                if q4 % 2 == 0:
                                    nc.vector.tensor_copy(Vt[:, g, :, 0:64], pv)
                                else:
                                    nc.scalar.activation(Vt[:, g, :, 0:64], pv,
                                                         AF.Copy)

                # ===== Phase 2: causal attention per (head, batch) ======
                def do_block(b, j):
                        n_kt = 4 * j + 4
                        PT = ptp.tile([128, 16, 2 * TPC], FP8, tag="pt")
                        for i in range(n_kt):
                            pss = aps.tile([128, 2 * TPC], F32, tag="ps2", bufs=2)
                            cb = 4 * b + i // 4
                            off = (i % 4) * 128
                            # diagonal tiles: queries below 128*d are fully
                            # masked, so skip them in the matmul/exp/mask and
                            # just zero that strip of PT
                            d = i - 4 * j
                            q0 = 128 * d if d > 0 else 0
                            PTv = PT[:, i, :].rearrange("p (h q) -> p h q",
                                                        h=HPC)
                            pssv = pss[:].rearrange("p (h q) -> p h q", h=HPC)
                            for h in range(HPC):
                                nc.tensor.matmul(
                                    pss[:, h * TPC + q0:(h + 1) * TPC],
                                    KT[64 * h:64 * h + 64, cb, off:off + 128],
                                    QT[64 * h:64 * h + 64, 4 * b + j, q0:],
                                    start=True, stop=True)
                            if q0:
                                nc.gpsimd.memset(PTv[:, :, 0:q0], 0.0)
                            nc.scalar.activation(PTv[:, :, q0:],
                                                 pssv[:, :, q0:],
                                                 AF.Exp, scale=0.125)
                            if i >= 4 * j:
                                eng = nc.vector if i % 2 == 0 else nc.gpsimd
                                eng.tensor_mul(
                                    PTv[:, :, q0:], PTv[:, :, q0:],
                                    trimask[:, d:d + 1, q0:].to_broadcast(
                                        [128, HPC, TPC - q0]))
                        for h in range(HPC):
                            ps_av = psA.tile([65, TPC], F32, tag="ps")
                            PTr = PT[:].rearrange("p i (h q) -> p i h q",
                                                  h=HPC)
                            for q in range(n_kt // 2):
                                nc.tensor.matmul(
                                    ps_av[:],
                                    Vt[:, 16 * b + 2 * q:16 * b + 2 * q + 2,
                                       h, 0:65],
                                    PTr[:, 2 * q:2 * q + 2, h, :],
                                    start=(q == 0), stop=(q == n_kt // 2 - 1),
                                    perf_mode=DR)
                            rec = avp.tile([1, TPC], F32, tag="avrec")
                            nc.vector.reciprocal(rec[:], ps_av[64:65, :])
                            recb = avp.tile([1, TPC], BF16, tag="avrecb")
                            nc.vector.tensor_copy(recb[:], rec[:])
                            rb = avp.tile([64, TPC], BF16, tag="avrb")
                            nc.gpsimd.partition_broadcast(rb[:], recb[:])
                            avn = avp.tile([64, TPC], FP8, tag="avn")
                            nc.vector.tensor_mul(avn[:], ps_av[0:64, :], rb[:])
                            nc.sync.dma_start(
                                a2ai[4 * b + j, 64 * h:64 * h + 64, :], avn[:])

                # batch-0 attention (needs only chunks 0-3) overlaps the
                # second half of the QKV streaming
                for tt in range(4):
                    do_chunk(tt)
                for j in range(4):
                    do_block(0, j)
                for tt in range(4, NT):
                    do_chunk(tt)
                xown = attnw.tile([128, PO, TPC], BF16)
                nc.sync.dma_start(xown[:], xo_in.ap())
                wo_sb = attnw.tile([128, PO, PO, 128], FP8)
                nc.sync.dma_start(wo_sb[:], wo_in.ap())
                for j in range(4):
                    do_block(1, j)

            # ==== Phase 3: AllToAll (overlapped with FF weight prefetch) ====
            nc.gpsimd.collective_compute(
                "AllToAll", ALU.bypass, replica_groups=RG,
                ins=[a2ai[:].opt()], outs=[a2ao[:].opt()])
            AVt = attnw.tile([128, NC_N, TPC], FP8)
            nc.sync.dma_start(AVt[:], a2ao[:].rearrange("s p t -> p s t"))

            # ========== Phase 4: output projection + residual ===============
            X2 = mlp.tile([128, PO, TPC], BF16)
            for m in range(PO):
                ps_o = psA.tile([128, TPC], F32, tag="ps")
                for q in range(PO // 2):
                    nc.tensor.matmul(ps_o[:], wo_sb[:, m, 2 * q:2 * q + 2, :],
                                     AVt[:, 2 * q:2 * q + 2, :],
                                     start=(q == 0), stop=(q == PO // 2 - 1),
                                     perf_mode=DR)
                to = rows.tile([128, TPC], BF16, tag="to")
                nc.scalar.activation(to[:], ps_o[:], AF.Identity,
                                     scale=1.0 / 32, bias=bo[:, m:m + 1])
                nc.vector.tensor_add(X2[:, m, :], to[:], xown[:, m, :])

        # ================= Phase 5: LN2-folded MLP ==========================
        A = mlp.tile([128, M1, TPC], FP8)
        with tc.tile_pool(name="w1p", bufs=4) as w1p, \
             tc.tile_pool(name="w2p", bufs=3) as w2p, \
             tc.tile_pool(name="sq2p", bufs=1) as sq2p, \
             tc.tile_pool(name="pst2", bufs=1, space="PSUM") as pstat2, \
             tc.tile_pool(name="psM", bufs=2, space="PSUM") as psM, \
             tc.tile_pool(name="gp", bufs=3) as gp, \
             tc.tile_pool(name="outp", bufs=3) as outp:
            sq2 = sq2p.tile([128, PO, TPC], BF16)
            for po in range(PO):
                if po % 2 == 0:
                    nc.gpsimd.tensor_mul(sq2[:, po, :], X2[:, po, :],
                                         X2[:, po, :])
                else:
                    nc.scalar.activation(sq2[:, po, :], X2[:, po, :],
                                         AF.Square)
            ps_st2 = pstat2.tile([1, TPC], F32, tag="st")
            ps_sq2 = pstat2.tile([1, TPC], F32, tag="stq")
            for po in range(PO):
                nc.tensor.matmul(ps_st2[0:1, :], invd_col_b[:], X2[:, po, :],
                                 start=(po == 0), stop=(po == PO - 1))
            for po in range(PO):
                nc.tensor.matmul(ps_sq2[0:1, :], invd_col_b[:], sq2[:, po, :],
                                 start=(po == 0), stop=(po == PO - 1))
            mu2_b, inv2_bc = row_stats(ps_st2, ps_sq2,
                                       inv_scale=1.0 / (32 * 32))
            mu2_bc = rows.tile([128, TPC], BF16, tag="mubc")
            nc.gpsimd.partition_broadcast(mu2_bc[:], mu2_b[:])
            # X2c = (X2 - mu2) * inv2 / 32  (the per-token LN2 scale rides the
            # matmul columns, so FF1 psum needs no per-token fixup at all)
            X2c = mlp.tile([128, PO, TPC], FP8)
            ct = gp.tile([128, PO, TPC], BF16, tag="ct")
            for po in range(PO):
                eng = nc.vector if po % 2 == 0 else nc.gpsimd
                eng.tensor_sub(ct[:, po, :], X2[:, po, :], mu2_bc[:])
                eng.tensor_mul(X2c[:, po, :], ct[:, po, :], inv2_bc[:])

            for m in range(M1):
                w1m = w1p.tile([128, PO, 128], FP8, tag="w1")
                nc.sync.dma_start(w1m[:], wf1_in.ap()[m])
                if m % 2 == 0:
                    ps1 = psA.tile([128, TPC], F32, tag="ps")
                else:
                    ps1 = psM.tile([128, TPC], F32, tag="psm")
                for q in range(PO // 2):
                    nc.tensor.matmul(ps1[:], w1m[:, 2 * q:2 * q + 2, :],
                                     X2c[:, 2 * q:2 * q + 2, :],
                                     start=(q == 0), stop=(q == PO // 2 - 1),
                                     perf_mode=DR)
                sig = gp.tile([128, TPC], BF16, tag="sig")
                nc.scalar.activation(sig[:], ps1[:], AF.Sigmoid,
                                     scale=SIG_A, bias=cf1s[:, m:m + 1])
                nc.vector.scalar_tensor_tensor(
                    out=A[:, m, :], in0=ps1[:], scalar=cf1[:, m:m + 1],
                    in1=sig[:], op0=ALU.add, op1=ALU.mult)

            out_view = out_t.ap().rearrange("(po p) t -> p po t", p=128)
            for m in range(PO):
                w2m = w2p.tile([128, M1, 128], FP8, tag="w2")
                nc.sync.dma_start(w2m[:], wf2_in.ap()[m])
                if m % 2 == 0:
                    ps_2 = psA.tile([128, TPC], F32, tag="ps")
                else:
                    ps_2 = psM.tile([128, TPC], F32, tag="psm")
                for q in range(M1 // 2):
                    nc.tensor.matmul(ps_2[:], w2m[:, 2 * q:2 * q + 2, :],
                                     A[:, 2 * q:2 * q + 2, :],
                                     start=(q == 0), stop=(q == M1 // 2 - 1),
                                     perf_mode=DR)
                t2 = gp.tile([128, TPC], F32, tag="t2")
                nc.scalar.activation(t2[:], ps_2[:], AF.Identity,
                                     scale=1.0 / 64, bias=cf2[:, m:m + 1])
                om = outp.tile([128, TPC], F32, tag="om")
                eng = nc.vector if m % 2 == 0 else nc.gpsimd
                eng.tensor_add(om[:], t2[:], X2[:, m, :])
                nc.sync.dma_start(out_view[:, m, :], om[:])

    nc.compile()
    return nc


def _get_nc():
    if "nc" not in _CACHE:
        _CACHE["nc"] = _build()
    return _CACHE["nc"]


def _make_in_maps(inputs):
    f32 = np.float32
    x = np.asarray(inputs["x"], f32).reshape(BT, D)
    ln1w = np.asarray(inputs["ln1_w"], f32)
    ln1b = np.asarray(inputs["ln1_b"], f32)
    ln2w = np.asarray(inputs["ln2_w"], f32)
    ln2b = np.asarray(inputs["ln2_b"], f32)
    W_qkv = np.asarray(inputs["W_qkv"], f32)
    b_qkv = np.asarray(inputs["b_qkv"], f32)
    W_o = np.asarray(inputs["W_o"], f32)
    b_o = np.asarray(inputs["b_o"], f32)
    W_ff1 = np.asarray(inputs["W_ff1"], f32)
    b_ff1 = np.asarray(inputs["b_ff1"], f32)
    W_ff2 = np.asarray(inputs["W_ff2"], f32)
    b_ff2 = np.asarray(inputs["b_ff2"], f32)

    # LN1 folded into QKV
    Wq_t = W_qkv * ln1w[:, None]            # [D, 3D]
    u_q = ln1w @ W_qkv                      # [3D]
    c_q = ln1b @ W_qkv + b_qkv              # [3D]
    # LN2 folded into FF1
    Wf1_t = W_ff1 * ln2w[:, None]           # [D, DFF]
    u_f = ln2w @ W_ff1                      # [DFF]
    c_f = ln2b @ W_ff1 + b_ff1              # [DFF]

    xt_full = np.ascontiguousarray(x.T).astype(BFNP)      # [D, BT]
    xt_full8 = xt_full.astype(F8NP)

    def pcol(v):  # [K*128] -> [128, K]
        return np.ascontiguousarray(v.reshape(-1, 128).T.astype(f32))

    # causal masks for the 4 diagonal sub-tiles: keep where tau >= 128*d + p
    p = np.arange(128)[:, None]
    tau = np.arange(TPC)[None, :]
    trimask = np.stack([(tau >= 128 * d + p) for d in range(4)],
                       axis=1).astype(F8NP)               # [128, 4, TPC]

    common = {
        "xt": xt_full8,
        "wo": np.ascontiguousarray(
            32.0 * W_o.reshape(PO, 128, PO, 128).transpose(1, 2, 0, 3)
        ).astype(F8NP),
        "bo": pcol(b_o),
        "wf1": np.ascontiguousarray(
            32.0 * Wf1_t.reshape(PO, 128, M1, 128).transpose(2, 1, 0, 3)
        ).astype(F8NP),
        "cf1": pcol(c_f),
        "cf1s": pcol(SIG_A * c_f),
        "wf2": np.ascontiguousarray(
            64.0 * W_ff2.reshape(M1, 128, PO, 128).transpose(2, 1, 0, 3)
        ).astype(F8NP),
        "cf2": pcol(b_ff2),
        "trimask": trimask,
    }
    in_maps = []
    for r in range(NC_N):
        hc = 128 * r          # first column of this core's Q/K/V head block
        m = dict(common)
        wqk = np.concatenate([Wq_t[:, hc:hc + 128],
                              Wq_t[:, D + hc:D + hc + 128]], axis=1)
        m["wqk8"] = np.ascontiguousarray(
            32.0 * wqk.reshape(PO, 128, 256).transpose(1, 0, 2)).astype(F8NP)
        m["wv8"] = np.ascontiguousarray(
            32.0 * Wq_t[:, 2 * D + hc:2 * D + hc + 128]
            .reshape(PO, 128, 128).transpose(1, 0, 2)).astype(F8NP)
        m["uqkv"] = np.ascontiguousarray(-32.0 * np.concatenate(
            [u_q[hc:hc + 128], u_q[D + hc:D + hc + 128],
             u_q[2 * D + hc:2 * D + hc + 128]])[None, :]).astype(BFNP)
        m["cqkv"] = np.ascontiguousarray(np.stack(
            [c_q[hc:hc + 128], c_q[D + hc:D + hc + 128],
             c_q[2 * D + hc:2 * D + hc + 128]], axis=1)).astype(f32)
        m["xo"] = np.ascontiguousarray(
            xt_full[:, TPC * r:TPC * (r + 1)].reshape(PO, 128, TPC)
            .transpose(1, 0, 2))
        in_maps.append(m)
    return in_maps


def _run(inputs, trace=False, trace_cores=None):
    nc = _get_nc()
    in_maps = _make_in_maps(inputs)
    res = bass_utils.run_bass_kernel_spmd(
        nc, in_maps, core_ids=list(range(NC_N)), trace=trace,
        trace_cores=trace_cores)
    outs = [res.results[r]["outt"] for r in range(NC_N)]
    full = np.concatenate([np.asarray(o, np.float32).T for o in outs], axis=0)
    return full.reshape(B, T, D).astype(np.float32), res


def kernel(**inputs):
    out, _ = _run(inputs, trace=False)
    return out
